# revision 15
# baseline (speedup 1.0000x reference)
"""Trainium2 Bass kernel for GQA attention (B=1, S=2048, D=2048, H=32, KV=8, HD=64).

Tensor-parallel over heads across 8 NeuronCores: core i holds q-heads
[4i, 4i+4) and kv-head i; each core computes its partial o_proj output and the
host sums the 8 partials (Megatron all-reduce done host-side).

v3: q-block-outer software pipeline. Per 512-row s-block: project q/k/v,
RoPE, 4 attention units, normalization, o_proj — with the previous block's
o_proj matmuls drip-fed between attention pairs so the tensor engine always
has fill work while softmax exp (scalar engine) is the per-pair critical path.

Self-contained: only imports concourse (on sys.path in the container).
"""

import os
import sys

import ml_dtypes
import numpy as np

if "/opt/trn_rl_repo" not in sys.path and not any(
    p.endswith("trn_rl_repo") for p in sys.path
):
    sys.path.insert(0, "/opt/trn_rl_repo")

import concourse.bass as bass
import concourse.mybir as mybir
import concourse.tile as tile
from concourse import bacc
from concourse.bass_utils import run_bass_kernel_spmd

F32 = mybir.dt.float32
BF16 = mybir.dt.bfloat16

AF = mybir.ActivationFunctionType
ALU = mybir.AluOpType

S = 2048
D = 2048
H = 32
KV = 8
HD = 64
NCORES = 8
HQ = H // NCORES  # 4 q heads per core
NSB = 4  # s blocks of 512
SBW = 512
DCH = D // 128  # 16 contraction chunks
VPAD = 80  # v_s chunk stride (bf16): 160 B, 32-byte aligned for xbar DMA


def _build_nc():
    nc = bacc.Bacc("TRN2", target_bir_lowering=False, debug=False, num_devices=NCORES)

    xt_d = nc.declare_dram_parameter("xt", [D, S], BF16, isOutput=False)
    wqkv_d = nc.declare_dram_parameter("wqkv", [D, 384], BF16, isOutput=False)
    wo_d = nc.declare_dram_parameter("wo", [2, 128, D], BF16, isOutput=False)
    cos_d = nc.declare_dram_parameter("cos", [128, S], BF16, isOutput=False)
    sin_d = nc.declare_dram_parameter("sin", [128, S], BF16, isOutput=False)
    sel_d = nc.declare_dram_parameter("sel", [16, 4 * 128], BF16, isOutput=False)
    y_d = nc.declare_dram_parameter("y", [S, D], BF16, isOutput=True)

    with tile.TileContext(nc) as tc:
        with tc.tile_pool(name="glob", bufs=1) as glob:
            ktdup = glob.tile([128, S], BF16, tag="ktdup")
            v_s = glob.tile([128, DCH, VPAD], BF16, tag="v_s")
            ao = glob.tile([128, 2, S], BF16, tag="ao")
            sel_s = glob.tile([16, 4 * 128], BF16, tag="sel_s")
            wo_s = glob.tile([128, 2, D], BF16, tag="wo_s")
            wq_s = glob.tile([128, DCH, 384], BF16, tag="wq_s")
            cos_s = glob.tile([128, S], BF16, tag="cos_s")
            sin_s = glob.tile([128, S], BF16, tag="sin_s")

            nc.vector.memset(v_s[:, :, 64], 1.0)  # softmax-sum ones column

            with (
                tc.tile_pool(name="xp", bufs=2) as xp,
                tc.tile_pool(name="kvp", bufs=2) as kvp,
                tc.tile_pool(name="abp", bufs=2) as abp,
                tc.tile_pool(name="qsp", bufs=5) as qsp,
                tc.tile_pool(name="tmpp", bufs=4) as tmpp,
                tc.tile_pool(name="ptp", bufs=6) as ptp,
                tc.tile_pool(name="stgp", bufs=6) as stgp,
                tc.tile_pool(name="smp", bufs=2) as smp,
                tc.tile_pool(name="yp", bufs=2) as yp,
                tc.tile_pool(name="ps1", bufs=2, space="PSUM") as ps1,
                tc.tile_pool(name="pssc", bufs=2, space="PSUM") as pssc,
                tc.tile_pool(name="pso_p", bufs=2, space="PSUM") as pso_p,
            ):
                xt_r = xt_d.rearrange("(ko p) s -> p ko s", p=128)
                wqkv_r = wqkv_d.rearrange("(ko p) n -> p ko n", p=128)

                stg_of = {}  # (h, sb) -> stg65 tile
                rcp_of = {}  # sb -> rcp_bf tile

                def emit_unit(h, sb, qs, sums_sb, pull):
                    """Attention unit for head h, q-block sb (512 q positions).

                    pull() is invoked after each chunk-pair to drip-feed other
                    tensor-engine work (prev block's o_proj) into the PE queue.
                    """
                    q0 = sb * SBW
                    nkc = 4 * (sb + 1)
                    pso = pso_p.tile([128, SBW], F32, tag="pso", name=f"pso_{h}_{sb}")
                    for pair in range(nkc // 2):
                        cA, cB = 2 * pair, 2 * pair + 1
                        psc = pssc.tile([128, 1024], F32, tag="psc", name=f"psc{h}{sb}{pair}")
                        ptt = ptp.tile([128, 1024], BF16, tag="ptt", name=f"ptt{h}{sb}{pair}")
                        for c, half, r0 in ((cA, 0, 0), (cB, 1, 64)):
                            kc0 = c * 128
                            d = max(0, kc0 - q0)
                            nc.tensor.matmul(
                                psc[:, half * 512 + d : half * 512 + 512],
                                lhsT=ktdup[r0 : r0 + 64, kc0 : kc0 + 128],
                                rhs=qs[r0 : r0 + 64, d:SBW],
                                start=True,
                                stop=True,
                                tile_position=(r0, 0),
                            )
                        dA = max(0, cA * 128 - q0)
                        dB = max(0, cB * 128 - q0)
                        # exp only over regions the pv matmuls read; the
                        # invalid 128-wide diagonal wedges are zeroed below.
                        if dB > 0:
                            nc.scalar.activation(
                                ptt[:, dA:512], psc[:, dA:512], AF.Exp
                            )
                            nc.scalar.activation(
                                ptt[:, 512 + dB : 1024], psc[:, 512 + dB : 1024], AF.Exp
                            )
                        else:
                            nc.scalar.activation(ptt[:, dA:1024], psc[:, dA:1024], AF.Exp)
                        for c, half, d in ((cA, 0, dA), (cB, 1, dB)):
                            kc0 = c * 128
                            if kc0 + 127 > q0:
                                # only the 128 columns crossing the diagonal
                                sl = slice(half * 512 + d, half * 512 + d + 128)
                                nc.gpsimd.affine_select(
                                    out=ptt[:, sl],
                                    in_=ptt[:, sl],
                                    compare_op=ALU.is_ge,
                                    fill=0.0,
                                    base=0,
                                    channel_multiplier=-1,
                                    pattern=[[1, 128]],
                                )
                        for c, half, d in ((cA, 0, dA), (cB, 1, dB)):
                            nc.tensor.matmul(
                                pso[0:65, d:SBW],
                                lhsT=v_s[:, c, 0:65],
                                rhs=ptt[:, half * 512 + d : half * 512 + 512],
                                start=(c == 0),
                                stop=(c == nkc - 1),
                            )
                        pull()
                    # evict raw attn out + sums in one copy; row 64 = softmax sums
                    stg = stgp.tile([65, SBW], F32, tag="stg", name=f"stg_{h}_{sb}")
                    nc.vector.tensor_copy(stg[:], pso[0:65, :])
                    stg_of[(h, sb)] = stg
                    if sums_sb is not None:
                        nc.gpsimd.dma_start(sums_sb[h : h + 1, :], stg[64:65, :])

                def normalize_a(sb, sums_sb):
                    """Reciprocal of softmax sums (vector engine chain)."""
                    rcp_f32 = smp.tile([16, SBW], F32, tag="rcp_f32", name=f"rf{sb}")
                    rcp_scr = smp.tile([16, SBW], F32, tag="rcp_scr", name=f"rs{sb}")
                    rcp_bf = smp.tile([16, SBW], BF16, tag="rcp_bf", name=f"rb{sb}")
                    nc.vector.reciprocal_approx_accurate(
                        rcp_f32[:], sums_sb[:], rcp_scr[:]
                    )
                    nc.vector.tensor_copy(rcp_bf[:], rcp_f32[:])
                    rcp_of[sb] = rcp_bf

                def norm_mult(h, sb, rcp_bf):
                    """Broadcast 1/sums row h via PE and scale into ao."""
                    ch = h // 2
                    rr = 64 * (h % 2)
                    sbc = slice(sb * SBW, (sb + 1) * SBW)
                    pbc = pso_p.tile([128, SBW], F32, tag="pso", name=f"pbc{h}{sb}")
                    nc.tensor.matmul(
                        pbc[:],
                        lhsT=sel_s[:, h * 128 : (h + 1) * 128],
                        rhs=rcp_bf[:],
                        start=True,
                        stop=True,
                    )
                    stg = stg_of.pop((h, sb))
                    nc.vector.tensor_tensor(
                        ao[rr : rr + 64, ch, sbc],
                        stg[0:64, :],
                        pbc[rr : rr + 64, :],
                        ALU.mult,
                    )

                def normalize_b(sb):
                    rcp_bf = rcp_of.pop(sb)
                    for h in range(HQ):
                        norm_mult(h, sb, rcp_bf)

                def o_proj_gen(psb):
                    """Yield-granulated o_proj of q-block psb (4 row-tiles)."""
                    for stl in range(4):
                        st = 4 * psb + stl
                        ysb = yp.tile([128, D], BF16, tag="ysb", name=f"ysb{st}")
                        for obp in range(2):
                            psys = [
                                ps1.tile(
                                    [128, SBW], F32, tag="proj", name=f"psy{st}{obp}{j}"
                                )
                                for j in range(2)
                            ]
                            for ch in range(2):
                                for j in range(2):
                                    ob = 2 * obp + j
                                    nc.tensor.matmul(
                                        psys[j][:],
                                        lhsT=ao[:, ch, st * 128 : (st + 1) * 128],
                                        rhs=wo_s[:, ch, ob * 512 : (ob + 1) * 512],
                                        start=(ch == 0),
                                        stop=(ch == 1),
                                    )
                                yield
                            for j in range(2):
                                osl = slice((2 * obp + j) * 512, (2 * obp + j + 1) * 512)
                                # scalar is idle while block 0's o_proj runs;
                                # later blocks overlap exp-heavy phases
                                if psb == 0 and j == 0:
                                    nc.scalar.activation(ysb[:, osl], psys[j][:], AF.Copy)
                                else:
                                    nc.vector.tensor_copy(ysb[:, osl], psys[j][:])
                            yield
                        eng = nc.gpsimd if st % 2 == 0 else nc.sync
                        eng.dma_start(y_d[st * 128 : (st + 1) * 128, :], ysb[:])
                        yield

                for sb in range(NSB):
                    sbc = slice(sb * SBW, (sb + 1) * SBW)
                    # reciprocal chain for previous block (runs during proj)
                    if sb > 0:
                        normalize_a(sb - 1, sums_prev)

                    # ---- projections for s-block sb ----
                    if sb == 0:
                        xblk = xp.tile([128, DCH, SBW], BF16, tag="xblk", name="xb0")
                        for kq in range(4):
                            for kc in range(4 * kq, 4 * kq + 4):
                                nc.sync.dma_start(wq_s[:, kc, :], wqkv_r[:, kc, :])
                            nc.sync.dma_start(
                                xblk[:, 4 * kq : 4 * kq + 4, :],
                                xt_r[:, 4 * kq : 4 * kq + 4, sbc],
                            )
                        # prefetch next x block before the lower-priority tables
                        nsbc = slice(SBW, 2 * SBW)
                        xblk_next = xp.tile(
                            [128, DCH, SBW], BF16, tag="xblk", name="xb1"
                        )
                        for kq in range(4):
                            nc.sync.dma_start(
                                xblk_next[:, 4 * kq : 4 * kq + 4, :],
                                xt_r[:, 4 * kq : 4 * kq + 4, nsbc],
                            )
                        nc.sync.dma_start(cos_s[:], cos_d[:])
                        nc.sync.dma_start(sin_s[:], sin_d[:])
                        nc.sync.dma_start(sel_s[:], sel_d[:])
                        # HAM warm-up: dummy matmuls keep the PE clock at full
                        # rate through the DMA-bound start (values unused).
                        wrm = pssc.tile([128, 1024], F32, tag="psc", name="wrm")
                        for w in range(40):
                            nc.tensor.matmul(
                                wrm[:, 0:512],
                                lhsT=ktdup[:, 0:128],
                                rhs=ktdup[:, 0:512],
                                start=True,
                                stop=True,
                            )
                        for ch in range(2):
                            nc.sync.dma_start(wo_s[:, ch, :], wo_d[ch])
                    else:
                        xblk = xblk_next

                    psKV = ps1.tile([128, SBW], F32, tag="proj", name=f"pKV{sb}")
                    psA = ps1.tile([128, SBW], F32, tag="proj", name=f"pA{sb}")
                    psB = ps1.tile([128, SBW], F32, tag="proj", name=f"pB{sb}")
                    for ps_t, col0 in ((psKV, 256), (psA, 0), (psB, 128)):
                        for kc in range(DCH):
                            nc.tensor.matmul(
                                ps_t[:],
                                lhsT=wq_s[:, kc, col0 : col0 + 128],
                                rhs=xblk[:, kc, :],
                                start=(kc == 0),
                                stop=(kc == DCH - 1),
                            )
                    # evict k|v and raw q to bf16 (scalar; idle during proj)
                    kvraw = kvp.tile([128, SBW], BF16, tag="kvraw", name=f"kv{sb}")
                    nc.scalar.activation(kvraw[:], psKV[:], AF.Copy)
                    qA_bf = abp.tile([128, SBW], BF16, tag="qA", name=f"qA{sb}")
                    qB_bf = abp.tile([128, SBW], BF16, tag="qB", name=f"qB{sb}")
                    nc.scalar.activation(qA_bf[:], psA[:], AF.Copy)
                    nc.scalar.activation(qB_bf[:], psB[:], AF.Copy)

                    # RoPE on the 4 q heads, all-bf16 (2x DVE rate)
                    outA = abp.tile([128, SBW], BF16, tag="outA", name=f"oA{sb}")
                    outB = abp.tile([128, SBW], BF16, tag="outB", name=f"oB{sb}")
                    tmp = tmpp.tile([128, SBW], BF16, tag="tmp", name=f"t1{sb}")
                    nc.vector.tensor_tensor(outA[:], qA_bf[:], cos_s[:, sbc], ALU.mult)
                    nc.vector.tensor_tensor(tmp[:], qB_bf[:], sin_s[:, sbc], ALU.mult)
                    nc.vector.tensor_tensor(outA[:], outA[:], tmp[:], ALU.subtract)
                    tmp2 = tmpp.tile([128, SBW], BF16, tag="tmp", name=f"t2{sb}")
                    nc.vector.tensor_tensor(outB[:], qB_bf[:], cos_s[:, sbc], ALU.mult)
                    nc.vector.tensor_tensor(tmp2[:], qA_bf[:], sin_s[:, sbc], ALU.mult)
                    nc.vector.tensor_tensor(outB[:], outB[:], tmp2[:], ALU.add)

                    # k RoPE on this s-block: kswap = [k_hi; k_lo]
                    kswap = kvp.tile([64, SBW], BF16, tag="kswap", name=f"ks{sb}")
                    nc.sync.dma_start(kswap[0:32, :], kvraw[32:64, :])
                    nc.sync.dma_start(kswap[32:64, :], kvraw[0:32, :])
                    nc.vector.tensor_tensor(
                        ktdup[0:64, sbc], kvraw[0:64, :], cos_s[0:64, sbc], ALU.mult
                    )
                    tmpk = tmpp.tile([64, SBW], BF16, tag="tmpk", name=f"tk{sb}")
                    nc.vector.tensor_tensor(tmpk[:], kswap[:], sin_s[0:64, sbc], ALU.mult)
                    nc.vector.tensor_tensor(
                        ktdup[0:32, sbc], ktdup[0:32, sbc], tmpk[0:32, :], ALU.subtract
                    )
                    nc.vector.tensor_tensor(
                        ktdup[32:64, sbc], ktdup[32:64, sbc], tmpk[32:64, :], ALU.add
                    )
                    nc.sync.dma_start(ktdup[64:128, sbc], ktdup[0:64, sbc])

                    # v: [64, 512] -> 4 key-chunk tiles [128, 64] via DMA xbar
                    for cl in range(4):
                        c = 4 * sb + cl
                        nc.sync.dma_start_transpose(
                            v_s[:, c, 0:64], kvraw[64:128, cl * 128 : (cl + 1) * 128]
                        )

                    # q streams for the 4 heads (duplicated rows for PE packing)
                    qs_h = []
                    for h in range(HQ):
                        hc = slice(32 * h, 32 * h + 32)
                        qs = qsp.tile([128, SBW], BF16, tag="qs", name=f"qs{h}_{sb}")
                        nc.vector.tensor_copy(qs[0:32, :], outA[hc, :])
                        nc.vector.tensor_copy(qs[32:64, :], outB[hc, :])
                        nc.vector.tensor_copy(qs[64:96, :], outA[hc, :])
                        nc.vector.tensor_copy(qs[96:128, :], outB[hc, :])
                        qs_h.append(qs)

                    # scale raw outputs of previous block into ao
                    if sb > 0:
                        normalize_b(sb - 1)

                    # prefetch next x block (issue before this block's units)
                    if 0 < sb < NSB - 1:
                        nsbc = slice((sb + 1) * SBW, (sb + 2) * SBW)
                        xblk_next = xp.tile(
                            [128, DCH, SBW], BF16, tag="xblk", name=f"xb{sb + 1}"
                        )
                        for kq in range(4):
                            nc.sync.dma_start(
                                xblk_next[:, 4 * kq : 4 * kq + 4, :],
                                xt_r[:, 4 * kq : 4 * kq + 4, nsbc],
                            )

                    # ---- attention units with prev-block o_proj drip-fed ----
                    gen = o_proj_gen(sb - 1) if sb > 0 else None
                    npairs = 8 * (sb + 1)
                    state = {"steps": 28, "pairs": npairs}  # 4*(4+2+1) gen yields

                    def pull(gen=gen, state=state):
                        if gen is None:
                            return
                        want = -(-state["steps"] // max(1, state["pairs"]))
                        state["pairs"] -= 1
                        for _ in range(want):
                            if next(gen, "done") == "done":
                                break
                            state["steps"] -= 1

                    sums_sb = smp.tile([16, SBW], F32, tag="sums", name=f"sums{sb}")
                    nc.vector.memset(sums_sb[:], 1.0)
                    if sb < NSB - 1:
                        for h in range(HQ):
                            emit_unit(h, sb, qs_h[h], sums_sb, pull)
                    else:
                        # last block: normalize heads 0-1 after unit 2 and
                        # heads 2-3 after unit 3, so only a short chain
                        # trails the final exp (base-0 slices only)
                        rcp_f32 = smp.tile([16, SBW], F32, tag="rcp_f32", name="rf3")
                        rcp_scr = smp.tile([16, SBW], F32, tag="rcp_scr", name="rs3")
                        rcp_bf = smp.tile([16, SBW], BF16, tag="rcp_bf", name="rb3")
                        nc.vector.memset(rcp_bf[:], 1.0)

                        def norm_pair(rows):
                            nc.vector.reciprocal_approx_accurate(
                                rcp_f32[0:rows, :],
                                sums_sb[0:rows, :],
                                rcp_scr[0:rows, :],
                            )
                            nc.vector.tensor_copy(
                                rcp_bf[0:rows, :], rcp_f32[0:rows, :]
                            )
                            for h in (rows - 2, rows - 1):
                                norm_mult(h, sb, rcp_bf)

                        for h in range(HQ):
                            emit_unit(h, sb, qs_h[h], sums_sb, pull)
                            if h == 2:
                                norm_pair(2)
                        norm_pair(4)
                    if gen is not None:
                        for _ in gen:
                            pass
                    sums_prev = sums_sb

                for _ in o_proj_gen(NSB - 1):
                    pass

    nc.compile()
    return nc


def _prep_inputs(x, Wq, Wk, Wv, Wo, inv_freq):
    """Host-side sharding + layout prep. Returns in_maps for the 8 cores."""
    x = np.ascontiguousarray(np.asarray(x, dtype=np.float32).reshape(S, D))
    xt = np.ascontiguousarray(x.T)  # [D, S]

    pos = np.arange(S, dtype=np.float64)
    inv = np.asarray(inv_freq, dtype=np.float64)  # [32]
    freqs = pos[None, :] * inv[:, None]  # [32, S]
    cos32 = np.cos(freqs).astype(np.float32)
    sin32 = np.sin(freqs).astype(np.float32)
    cos_tab = np.tile(cos32, (4, 1))  # [128, S]
    sin_tab = np.tile(sin32, (4, 1))
    # sel[h, h*128 + 64*(h%2) : +64] = 1 broadcasts rcp row h to the ao rows
    # of head h (chunk h//2, row offset 64*(h%2)).
    sel = np.zeros((16, 4 * 128), dtype=np.float32)
    for h in range(HQ):
        rr = 64 * (h % 2)
        sel[h, h * 128 + rr : h * 128 + rr + 64] = 1.0

    in_maps = []
    for i in range(NCORES):
        wq_l = Wq[256 * i : 256 * (i + 1)].astype(np.float32) * 0.125  # [256, D]
        wk_l = Wk[64 * i : 64 * (i + 1)].astype(np.float32)  # [64, D]
        wv_l = Wv[64 * i : 64 * (i + 1)].astype(np.float32)  # [64, D]
        # A-tile: first-half dims of the 4 heads; B-tile: second halves
        wA = np.concatenate(
            [wq_l[64 * h : 64 * h + 32] for h in range(HQ)], axis=0
        )  # [128, D]
        wB = np.concatenate(
            [wq_l[64 * h + 32 : 64 * h + 64] for h in range(HQ)], axis=0
        )
        wkv = np.concatenate([wk_l, wv_l], axis=0)  # [128, D]
        wqkv = np.ascontiguousarray(
            np.concatenate([wA, wB, wkv], axis=0).T
        )  # [D, 384]
        wo_l = Wo[:, 256 * i : 256 * (i + 1)].astype(np.float32)  # [D, 256]
        wo_t = np.ascontiguousarray(wo_l.T.reshape(2, 128, D))  # [2, 128, D]
        in_maps.append(
            {
                "xt": xt.astype(ml_dtypes.bfloat16),
                "wqkv": wqkv.astype(ml_dtypes.bfloat16),
                "wo": wo_t.astype(ml_dtypes.bfloat16),
                "cos": cos_tab.astype(ml_dtypes.bfloat16),
                "sin": sin_tab.astype(ml_dtypes.bfloat16),
                "sel": sel.astype(ml_dtypes.bfloat16),
            }
        )
    return in_maps


_NC_CACHE = None


def kernel(x, Wq, Wk, Wv, Wo, inv_freq):
    global _NC_CACHE
    if _NC_CACHE is None:
        _NC_CACHE = _build_nc()
    nc = _NC_CACHE
    in_maps = _prep_inputs(x, Wq, Wk, Wv, Wo, inv_freq)
    trace = bool(int(os.environ.get("BASS_KERNEL_TRACE", "0")))
    res = None
    last_exc = None
    for attempt in range(3):
        try:
            res = run_bass_kernel_spmd(nc, in_maps, list(range(NCORES)), trace=trace)
            break
        except Exception as e:  # transient device faults (rare) — retry
            last_exc = e
            msg = str(e)
            if "UNRECOVERABLE" in msg or "UNAVAILABLE" in msg or "Timeout" in msg:
                continue
            raise
    if res is None:
        raise last_exc
    if trace:
        kernel.last_results = res
    y = np.zeros((S, D), dtype=np.float32)
    for i in range(NCORES):
        y += res.results[i]["y"].astype(np.float32)
    return y.reshape(1, S, D)


# revision 19
# speedup vs baseline: 1.0042x; 1.0042x over previous
"""Trainium2 Bass kernel for GQA attention (B=1, S=2048, D=2048, H=32, KV=8, HD=64).

Tensor-parallel over heads across 8 NeuronCores: core i holds q-heads
[4i, 4i+4) and kv-head i; each core computes its partial o_proj output and the
host sums the 8 partials (Megatron all-reduce done host-side).

Self-contained: only imports concourse (on sys.path in the container).
"""

import os
import sys

import ml_dtypes
import numpy as np

if "/opt/trn_rl_repo" not in sys.path and not any(
    p.endswith("trn_rl_repo") for p in sys.path
):
    sys.path.insert(0, "/opt/trn_rl_repo")

import concourse.bass as bass
import concourse.mybir as mybir
import concourse.tile as tile
from concourse import bacc
from concourse.bass_utils import run_bass_kernel_spmd
from concourse.masks import make_identity

F32 = mybir.dt.float32
F32R = mybir.dt.float32r
BF16 = mybir.dt.bfloat16


def _r(ap):
    return ap.bitcast(F32R)
AF = mybir.ActivationFunctionType
ALU = mybir.AluOpType

S = 2048
D = 2048
H = 32
KV = 8
HD = 64
NCORES = 8
HQ = H // NCORES  # 4 q heads per core
NKC = S // 128  # 16 key chunks
NQB = 4  # q blocks of 512
QBW = 512
NSB = 4  # s blocks of 512 in projection
SBW = 512
DCH = D // 128  # 16 contraction chunks

def _build_nc():
    nc = bacc.Bacc("TRN2", target_bir_lowering=False, debug=False, num_devices=NCORES)

    xt_d = nc.declare_dram_parameter("xt", [D, S], BF16, isOutput=False)
    wqkv_d = nc.declare_dram_parameter("wqkv", [D, 384], BF16, isOutput=False)
    wo_d = nc.declare_dram_parameter("wo", [2, 128, D], BF16, isOutput=False)
    cos_d = nc.declare_dram_parameter("cos", [128, S], BF16, isOutput=False)
    sin_d = nc.declare_dram_parameter("sin", [128, S], BF16, isOutput=False)
    sel_d = nc.declare_dram_parameter("sel", [16, 16 * 128], BF16, isOutput=False)
    y_d = nc.declare_dram_parameter("y", [S, D], BF16, isOutput=True)

    with tile.TileContext(nc) as tc:
        with (
            tc.tile_pool(name="glob", bufs=1) as glob,
            tc.tile_pool(name="globp", bufs=2, space="PSUM") as globp,
        ):
            ktdup = glob.tile([128, S], BF16, tag="ktdup")
            v_s = glob.tile([128, NKC, 65], BF16, tag="v_s")
            outA = glob.tile([128, S], BF16, tag="outA")
            outB = glob.tile([128, S], BF16, tag="outB")
            ao = glob.tile([128, 2, S], BF16, tag="ao")
            sel_s = glob.tile([16, 16 * 128], BF16, tag="sel_s")
            ident = glob.tile([128, 128], F32, tag="ident")
            wo_s = glob.tile([128, 2, D], BF16, tag="wo_s")
            sums_h = [
                glob.tile([16, QBW], F32, tag="sums_h", name=f"sums_h{i}")
                for i in range(HQ)
            ]
            rcp_all = glob.tile([16, QBW], BF16, tag="rcp_all")
            rcp_f32 = glob.tile([16, QBW], F32, tag="rcp_f32")
            rcp_scr = glob.tile([16, QBW], F32, tag="rcp_scr")

            nc.vector.memset(v_s[:, :, 64], 1.0)
            for t in sums_h:
                nc.vector.memset(t[:], 1.0)

            # ------------- unified pools: P1/P2/P3 share PSUM explicitly -------
            with (
                tc.tile_pool(name="p1", bufs=1) as p1,
                tc.tile_pool(name="xp", bufs=3) as xp,
                tc.tile_pool(name="tmpp", bufs=4) as tmpp,
                tc.tile_pool(name="qsp", bufs=4) as qsp,
                tc.tile_pool(name="ptp", bufs=10) as ptp,
                tc.tile_pool(name="stgp", bufs=4) as stgp,
                tc.tile_pool(name="yp", bufs=8) as yp,
                tc.tile_pool(name="ps1", bufs=2, space="PSUM") as ps1,
                tc.tile_pool(name="pssc", bufs=2, space="PSUM") as pssc,
                tc.tile_pool(name="pso_p", bufs=2, space="PSUM") as pso_p,
            ):
                # ---------------- P1: projections + RoPE + v transpose ---------
                wq_s = p1.tile([128, DCH, 384], BF16, tag="wq_s")
                wqkv_r = wqkv_d.rearrange("(ko p) n -> p ko n", p=128)
                cos_s = p1.tile([128, S], BF16, tag="cos_s")
                sin_s = p1.tile([128, S], BF16, tag="sin_s")
                kvraw = p1.tile([128, S], F32, tag="kvraw")
                kswap = p1.tile([64, S], F32, tag="kswap")

                def emit_unit(h, qb, qs):
                    q0 = qb * QBW
                    nkc = 4 * (qb + 1)
                    pso = pso_p.tile([128, QBW], F32, tag="pso")
                    for pair in range(nkc // 2):
                        cA, cB = 2 * pair, 2 * pair + 1
                        psc = pssc.tile([128, 1024], F32, tag="psc")
                        ptt = ptp.tile([128, 1024], BF16, tag="ptt")
                        for c, half, r0 in ((cA, 0, 0), (cB, 1, 64)):
                            kc0 = c * 128
                            d = max(0, kc0 - q0)
                            nc.tensor.matmul(
                                psc[:, half * 512 + d : half * 512 + 512],
                                lhsT=ktdup[r0 : r0 + 64, kc0 : kc0 + 128],
                                rhs=qs[r0 : r0 + 64, q0 + d : q0 + QBW],
                                start=True,
                                stop=True,
                                tile_position=(r0, 0),
                            )
                        dA = max(0, cA * 128 - q0)
                        dB = max(0, cB * 128 - q0)
                        # exp only over the regions the pv matmuls read; the
                        # 128-wide diagonal wedges are zeroed below.
                        if dB > 0:
                            nc.scalar.activation(
                                ptt[:, dA:512], psc[:, dA:512], AF.Exp
                            )
                            nc.scalar.activation(
                                ptt[:, 512 + dB : 1024], psc[:, 512 + dB : 1024], AF.Exp
                            )
                        else:
                            nc.scalar.activation(
                                ptt[:, dA:1024], psc[:, dA:1024], AF.Exp
                            )
                        for c, half, d in ((cA, 0, dA), (cB, 1, dB)):
                            kc0 = c * 128
                            if kc0 + 127 > q0:
                                # only the 128 columns crossing the diagonal
                                sl = slice(half * 512 + d, half * 512 + d + 128)
                                nc.gpsimd.affine_select(
                                    out=ptt[:, sl],
                                    in_=ptt[:, sl],
                                    compare_op=ALU.is_ge,
                                    fill=0.0,
                                    base=0,
                                    channel_multiplier=-1,
                                    pattern=[[1, 128]],
                                )
                        for c, half, d in ((cA, 0, dA), (cB, 1, dB)):
                            nc.tensor.matmul(
                                pso[0:65, d:QBW],
                                lhsT=v_s[:, c, :],
                                rhs=ptt[:, half * 512 + d : half * 512 + 512],
                                start=(c == 0),
                                stop=(c == nkc - 1),
                            )
                    # evict raw attn out + sums
                    ch = h // 2
                    rr = 64 * (h % 2)
                    stg = stgp.tile([128, QBW], BF16, tag="stg")
                    nc.vector.tensor_copy(stg[0:64, :], pso[0:64, :])
                    sumr = stgp.tile([128, QBW], F32, tag="sumr")
                    nc.vector.tensor_copy(sumr[64:65, :], pso[64:65, :])
                    nc.sync.dma_start(
                        ao[rr : rr + 64, ch, qb * QBW : (qb + 1) * QBW], stg[0:64, :]
                    )
                    nc.gpsimd.dma_start(
                        sums_h[h][qb : qb + 1, :], sumr[64:65, :]
                    )

                xt_r = xt_d.rearrange("(ko p) s -> p ko s", p=128)
                # HAM warm-up: dummy matmuls keep the PE clock ramping to full
                # rate through the DMA-bound start (values unused).
                wrm = pso_p.tile([128, QBW], F32, tag="pso", name="wrm")
                for w in range(20):
                    nc.tensor.matmul(
                        wrm[:],
                        lhsT=ktdup[:, 0:128],
                        rhs=ktdup[:, 0:512],
                        start=True,
                        stop=True,
                    )
                qs_all = [qsp.tile([128, S], BF16, tag="qs", name=f"qs{i}") for i in range(HQ)]
                qs0 = qs_all[0]
                for sb in range(NSB):
                    sbc = slice(sb * SBW, (sb + 1) * SBW)
                    xblk = xp.tile([128, DCH, SBW], BF16, tag="xblk")
                    for kq in range(4):
                        if sb == 0:
                            for kc in range(4 * kq, 4 * kq + 4):
                                nc.sync.dma_start(wq_s[:, kc, :], wqkv_r[:, kc, :])
                        nc.sync.dma_start(
                            xblk[:, 4 * kq : 4 * kq + 4, :],
                            xt_r[:, 4 * kq : 4 * kq + 4, sbc],
                        )
                    if sb == 0:
                        nc.sync.dma_start(cos_s[:], cos_d[:])
                        nc.sync.dma_start(sin_s[:], sin_d[:])
                        make_identity(nc, ident[:])
                    psKV = ps1.tile([128, SBW], F32, tag="proj")
                    psA = ps1.tile([128, SBW], F32, tag="proj")
                    psB = ps1.tile([128, SBW], F32, tag="proj")
                    for ps_t, col0 in ((psKV, 256), (psA, 0), (psB, 128)):
                        for kc in range(DCH):
                            nc.tensor.matmul(
                                ps_t[:],
                                lhsT=wq_s[:, kc, col0 : col0 + 128],
                                rhs=xblk[:, kc, :],
                                start=(kc == 0),
                                stop=(kc == DCH - 1),
                            )
                    # evict k|v rows early (frees the KV slot)
                    nc.scalar.activation(kvraw[:, sbc], psKV[:], AF.Copy)

                    # RoPE on the 4 q heads (A = first-half dims, B = second)
                    tmp = tmpp.tile([128, SBW], F32, tag="tmp")
                    nc.vector.tensor_tensor(
                        outA[:, sbc], psA[:], cos_s[:, sbc], ALU.mult
                    )
                    nc.vector.tensor_tensor(tmp[:], psB[:], sin_s[:, sbc], ALU.mult)
                    nc.vector.tensor_tensor(
                        outA[:, sbc], outA[:, sbc], tmp[:], ALU.subtract
                    )
                    tmp2 = tmpp.tile([128, SBW], F32, tag="tmp")
                    nc.vector.tensor_tensor(
                        outB[:, sbc], psB[:], cos_s[:, sbc], ALU.mult
                    )
                    nc.vector.tensor_tensor(tmp2[:], psA[:], sin_s[:, sbc], ALU.mult)
                    nc.vector.tensor_tensor(
                        outB[:, sbc], outB[:, sbc], tmp2[:], ALU.add
                    )

                    # k RoPE on this s-block: kswap = [k_hi; k_lo]
                    nc.sync.dma_start(kswap[0:32, sbc], kvraw[32:64, sbc])
                    nc.sync.dma_start(kswap[32:64, sbc], kvraw[0:32, sbc])
                    nc.vector.tensor_tensor(
                        ktdup[0:64, sbc], kvraw[0:64, sbc], cos_s[0:64, sbc], ALU.mult
                    )
                    tmpk = tmpp.tile([64, SBW], F32, tag="tmpk")
                    nc.vector.tensor_tensor(
                        tmpk[:], kswap[:, sbc], sin_s[0:64, sbc], ALU.mult
                    )
                    nc.vector.tensor_tensor(
                        ktdup[0:32, sbc], ktdup[0:32, sbc], tmpk[0:32, :],
                        ALU.subtract,
                    )
                    nc.vector.tensor_tensor(
                        ktdup[32:64, sbc], ktdup[32:64, sbc], tmpk[32:64, :],
                        ALU.add,
                    )
                    nc.sync.dma_start(ktdup[64:128, sbc], ktdup[0:64, sbc])

                    # v: [64, 512] -> 4 key-chunk tiles [128, 64] via PE transpose
                    for c in range(4 * sb, 4 * sb + 4):
                        ptr = pso_p.tile([128, QBW], F32, tag="pso")
                        nc.tensor.transpose(
                            ptr[:, 0:64],
                            kvraw[64:128, c * 128 : (c + 1) * 128],
                            ident[64:128, 64:128],
                        )
                        nc.vector.tensor_copy(v_s[:, c, 0:64], ptr[:, 0:64])
                    # head-0 q stream for this s-block + interleaved unit
                    hc0 = slice(0, 32)
                    nc.sync.dma_start(qs0[0:32, sbc], outA[hc0, sbc])
                    nc.sync.dma_start(qs0[32:64, sbc], outB[hc0, sbc])
                    nc.sync.dma_start(qs0[64:96, sbc], outA[hc0, sbc])
                    nc.sync.dma_start(qs0[96:128, sbc], outB[hc0, sbc])
                    if sb >= 1:
                        emit_unit(0, sb - 1, qs0)

                # ---------------- P2: attention --------------------------------
                nc.sync.dma_start(sel_s[:], sel_d[:])
                for ch in range(2):
                    nc.sync.dma_start(wo_s[:, ch, :], wo_d[ch])

                def _defer_normalize(hh):
                    nc.vector.reciprocal_approx_accurate(
                        rcp_f32[:], sums_h[hh][:], rcp_scr[:]
                    )
                    nc.vector.tensor_copy(rcp_all[:], rcp_f32[:])
                    for qb2 in range(NQB):
                        idx2 = 2 * qb2 + (hh % 2)
                        ch2 = hh // 2
                        rr2 = 64 * (hh % 2)
                        q02 = qb2 * QBW
                        pbc = pssc.tile([128, 1024], F32, tag="psc")
                        nc.tensor.matmul(
                            pbc[:, 0:QBW],
                            lhsT=sel_s[:, idx2 * 128 : (idx2 + 1) * 128],
                            rhs=rcp_all[:],
                            start=True,
                            stop=True,
                        )
                        nc.vector.tensor_tensor(
                            ao[rr2 : rr2 + 64, ch2, q02 : q02 + QBW],
                            ao[rr2 : rr2 + 64, ch2, q02 : q02 + QBW],
                            pbc[rr2 : rr2 + 64, 0:QBW],
                            ALU.mult,
                        )

                def _normalize_one(hh, qb2):
                    nc.vector.reciprocal_approx_accurate(
                        rcp_f32[:], sums_all[:], rcp_scr[:]
                    )
                    nc.vector.tensor_copy(rcp_all[:], rcp_f32[:])
                    idx2 = 4 * hh + qb2
                    ch2 = hh // 2
                    rr2 = 64 * (hh % 2)
                    q02 = qb2 * QBW
                    pbc = pssc.tile([128, 1024], F32, tag="psc")
                    nc.tensor.matmul(
                        pbc[:, 0:QBW],
                        lhsT=sel_s[:, idx2 * 128 : (idx2 + 1) * 128],
                        rhs=rcp_all[:],
                        start=True,
                        stop=True,
                    )
                    nc.vector.tensor_tensor(
                        ao[rr2 : rr2 + 64, ch2, q02 : q02 + QBW],
                        ao[rr2 : rr2 + 64, ch2, q02 : q02 + QBW],
                        pbc[rr2 : rr2 + 64, 0:QBW],
                        ALU.mult,
                    )

                for h in range(HQ):
                    if h == 0:
                        emit_unit(0, 3, qs0)
                        continue
                    hc = slice(32 * h, 32 * h + 32)
                    qs = qs_all[h]
                    for sb in range(NSB):
                        sbc = slice(sb * SBW, (sb + 1) * SBW)
                        nc.sync.dma_start(qs[0:32, sbc], outA[hc, sbc])
                        nc.sync.dma_start(qs[32:64, sbc], outB[hc, sbc])
                        nc.sync.dma_start(qs[64:96, sbc], outA[hc, sbc])
                        nc.sync.dma_start(qs[96:128, sbc], outB[hc, sbc])
                    qbo = (3, 2, 1, 0) if h == HQ - 1 else range(NQB)
                    for qb in qbo:
                        emit_unit(h, qb, qs)
                    _defer_normalize(h - 1)
                    if h == HQ - 1:
                        _defer_normalize(h)

                # ---------------- P3: o_proj -----------------------------------
                for st in range(16):
                    for obp in range(2):
                        psy = pssc.tile([128, 1024], F32, tag="psc")
                        for oh in range(2):
                            ob = 2 * obp + oh
                            for ch in range(2):
                                nc.tensor.matmul(
                                    psy[:, oh * 512 : (oh + 1) * 512],
                                    lhsT=ao[:, ch, st * 128 : (st + 1) * 128],
                                    rhs=wo_s[:, ch, ob * 512 : (ob + 1) * 512],
                                    start=(ch == 0),
                                    stop=(ch == 1),
                                )
                        ysb = yp.tile([128, 1024], BF16, tag="ysb")
                        # split eviction across both engines so it never
                        # falls behind the two matmuls per psy tile
                        nc.scalar.activation(ysb[:, 0:512], psy[:, 0:512], AF.Copy)
                        nc.vector.tensor_copy(ysb[:, 512:1024], psy[:, 512:1024])
                        eng = nc.gpsimd if st % 2 == 0 else nc.sync
                        eng.dma_start(
                            y_d[
                                st * 128 : (st + 1) * 128,
                                obp * 1024 : (obp + 1) * 1024,
                            ],
                            ysb[:],
                        )
    nc.compile()
    return nc


def _prep_inputs(x, Wq, Wk, Wv, Wo, inv_freq):
    """Host-side sharding + layout prep. Returns in_maps for the 8 cores."""
    x = np.ascontiguousarray(np.asarray(x, dtype=np.float32).reshape(S, D))
    xt = np.ascontiguousarray(x.T)  # [D, S]

    pos = np.arange(S, dtype=np.float64)
    inv = np.asarray(inv_freq, dtype=np.float64)  # [32]
    freqs = pos[None, :] * inv[:, None]  # [32, S]
    cos32 = np.cos(freqs).astype(np.float32)
    sin32 = np.sin(freqs).astype(np.float32)
    cos_tab = np.tile(cos32, (4, 1))  # [128, S]
    sin_tab = np.tile(sin32, (4, 1))
    sel = np.zeros((16, 16 * 128), dtype=np.float32)
    for qb in range(4):
        for par in range(2):
            blk = 2 * qb + par
            rr = 64 * par
            sel[qb, blk * 128 + rr : blk * 128 + rr + 64] = 1.0

    in_maps = []
    for i in range(NCORES):
        wq_l = Wq[256 * i : 256 * (i + 1)].astype(np.float32) * 0.125  # [256, D]
        wk_l = Wk[64 * i : 64 * (i + 1)].astype(np.float32)  # [64, D]
        wv_l = Wv[64 * i : 64 * (i + 1)].astype(np.float32)  # [64, D]
        # A-tile: first-half dims of the 4 heads; B-tile: second halves
        wA = np.concatenate(
            [wq_l[64 * h : 64 * h + 32] for h in range(HQ)], axis=0
        )  # [128, D]
        wB = np.concatenate(
            [wq_l[64 * h + 32 : 64 * h + 64] for h in range(HQ)], axis=0
        )
        wkv = np.concatenate([wk_l, wv_l], axis=0)  # [128, D]
        wqkv = np.ascontiguousarray(
            np.concatenate([wA, wB, wkv], axis=0).T
        )  # [D, 384]
        wo_l = Wo[:, 256 * i : 256 * (i + 1)].astype(np.float32)  # [D, 256]
        wo_t = np.ascontiguousarray(wo_l.T.reshape(2, 128, D))  # [2, 128, D]
        in_maps.append(
            {
                "xt": xt.astype(ml_dtypes.bfloat16),
                "wqkv": wqkv.astype(ml_dtypes.bfloat16),
                "wo": wo_t.astype(ml_dtypes.bfloat16),
                "cos": cos_tab.astype(ml_dtypes.bfloat16),
                "sin": sin_tab.astype(ml_dtypes.bfloat16),
                "sel": sel.astype(ml_dtypes.bfloat16),
            }
        )
    return in_maps


_NC_CACHE = None


def kernel(x, Wq, Wk, Wv, Wo, inv_freq):
    global _NC_CACHE
    if _NC_CACHE is None:
        _NC_CACHE = _build_nc()
    nc = _NC_CACHE
    in_maps = _prep_inputs(x, Wq, Wk, Wv, Wo, inv_freq)
    trace = bool(int(os.environ.get("BASS_KERNEL_TRACE", "0")))
    res = None
    last_exc = None
    for attempt in range(3):
        try:
            res = run_bass_kernel_spmd(nc, in_maps, list(range(NCORES)), trace=trace)
            break
        except Exception as e:  # transient device faults (rare) — retry
            last_exc = e
            msg = str(e)
            if "UNRECOVERABLE" in msg or "UNAVAILABLE" in msg or "Timeout" in msg:
                continue
            raise
    if res is None:
        raise last_exc
    if trace:
        kernel.last_results = res
    y = np.zeros((S, D), dtype=np.float32)
    for i in range(NCORES):
        y += res.results[i]["y"].astype(np.float32)
    return y.reshape(1, S, D)



# revision 20
# speedup vs baseline: 1.0286x; 1.0243x over previous
"""Trainium2 Bass kernel for GQA attention (B=1, S=2048, D=2048, H=32, KV=8, HD=64).

Tensor-parallel over heads across 8 NeuronCores: core i holds q-heads
[4i, 4i+4) and kv-head i; each core computes its partial o_proj output and the
host sums the 8 partials (Megatron all-reduce done host-side).

Self-contained: only imports concourse (on sys.path in the container).
"""

import os
import sys

import ml_dtypes
import numpy as np

if "/opt/trn_rl_repo" not in sys.path and not any(
    p.endswith("trn_rl_repo") for p in sys.path
):
    sys.path.insert(0, "/opt/trn_rl_repo")

import concourse.bass as bass
import concourse.mybir as mybir
import concourse.tile as tile
from concourse import bacc
from concourse.bass_utils import run_bass_kernel_spmd
from concourse.masks import make_identity

F32 = mybir.dt.float32
F32R = mybir.dt.float32r
BF16 = mybir.dt.bfloat16


def _r(ap):
    return ap.bitcast(F32R)
AF = mybir.ActivationFunctionType
ALU = mybir.AluOpType

S = 2048
D = 2048
H = 32
KV = 8
HD = 64
NCORES = 8
HQ = H // NCORES  # 4 q heads per core
NKC = S // 128  # 16 key chunks
NQB = 4  # q blocks of 512
QBW = 512
NSB = 4  # s blocks of 512 in projection
SBW = 512
DCH = D // 128  # 16 contraction chunks

def _build_nc():
    nc = bacc.Bacc("TRN2", target_bir_lowering=False, debug=False, num_devices=NCORES)

    xt_d = nc.declare_dram_parameter("xt", [D, S], BF16, isOutput=False)
    wqkv_d = nc.declare_dram_parameter("wqkv", [D, 384], BF16, isOutput=False)
    wo_d = nc.declare_dram_parameter("wo", [2, 128, D], BF16, isOutput=False)
    cos_d = nc.declare_dram_parameter("cos", [128, S], BF16, isOutput=False)
    sin_d = nc.declare_dram_parameter("sin", [128, S], BF16, isOutput=False)
    sel_d = nc.declare_dram_parameter("sel", [16, 16 * 128], BF16, isOutput=False)
    y_d = nc.declare_dram_parameter("y", [S, D], BF16, isOutput=True)

    with tile.TileContext(nc) as tc:
        with (
            tc.tile_pool(name="glob", bufs=1) as glob,
            tc.tile_pool(name="globp", bufs=2, space="PSUM") as globp,
        ):
            ktdup = glob.tile([128, S], BF16, tag="ktdup")
            v_s = glob.tile([128, NKC, 65], BF16, tag="v_s")
            outA = glob.tile([128, S], BF16, tag="outA")
            outB = glob.tile([128, S], BF16, tag="outB")
            ao = glob.tile([128, 2, S], BF16, tag="ao")
            sel_s = glob.tile([16, 16 * 128], BF16, tag="sel_s")
            ident = glob.tile([128, 128], F32, tag="ident")
            wo_s = glob.tile([128, 2, D], BF16, tag="wo_s")
            sums_h = [
                glob.tile([16, QBW], F32, tag="sums_h", name=f"sums_h{i}")
                for i in range(HQ)
            ]
            rcp_all = glob.tile([16, QBW], BF16, tag="rcp_all")
            rcp_f32 = glob.tile([16, QBW], F32, tag="rcp_f32")
            rcp_scr = glob.tile([16, QBW], F32, tag="rcp_scr")

            nc.vector.memset(v_s[:, :, 64], 1.0)
            for t in sums_h:
                nc.vector.memset(t[:], 1.0)

            # ------------- unified pools: P1/P2/P3 share PSUM explicitly -------
            with (
                tc.tile_pool(name="p1", bufs=1) as p1,
                tc.tile_pool(name="xp", bufs=3) as xp,
                tc.tile_pool(name="tmpp", bufs=4) as tmpp,
                tc.tile_pool(name="qsp", bufs=4) as qsp,
                tc.tile_pool(name="ptp", bufs=10) as ptp,
                tc.tile_pool(name="stgp", bufs=4) as stgp,
                tc.tile_pool(name="yp", bufs=8) as yp,
                tc.tile_pool(name="ps1", bufs=2, space="PSUM") as ps1,
                tc.tile_pool(name="pssc", bufs=2, space="PSUM") as pssc,
                tc.tile_pool(name="pso_p", bufs=2, space="PSUM") as pso_p,
            ):
                # ---------------- P1: projections + RoPE + v transpose ---------
                wq_s = p1.tile([128, DCH, 384], BF16, tag="wq_s")
                wqkv_r = wqkv_d.rearrange("(ko p) n -> p ko n", p=128)
                cos_s = p1.tile([128, S], BF16, tag="cos_s")
                sin_s = p1.tile([128, S], BF16, tag="sin_s")
                kvraw = p1.tile([128, S], F32, tag="kvraw")
                kswap = p1.tile([64, S], F32, tag="kswap")

                def emit_unit(h, qb, qs):
                    q0 = qb * QBW
                    nkc = 4 * (qb + 1)
                    pso = pso_p.tile([128, QBW], F32, tag="pso")
                    for pair in range(nkc // 2):
                        cA, cB = 2 * pair, 2 * pair + 1
                        psc = pssc.tile([128, 1024], F32, tag="psc")
                        ptt = ptp.tile([128, 1024], BF16, tag="ptt")
                        for c, half, r0 in ((cA, 0, 0), (cB, 1, 64)):
                            kc0 = c * 128
                            d = max(0, kc0 - q0)
                            nc.tensor.matmul(
                                psc[:, half * 512 + d : half * 512 + 512],
                                lhsT=ktdup[r0 : r0 + 64, kc0 : kc0 + 128],
                                rhs=qs[r0 : r0 + 64, q0 + d : q0 + QBW],
                                start=True,
                                stop=True,
                                tile_position=(r0, 0),
                            )
                        dA = max(0, cA * 128 - q0)
                        dB = max(0, cB * 128 - q0)
                        # exp only over the regions the pv matmuls read; the
                        # 128-wide diagonal wedges are zeroed below.
                        if dB > 0:
                            nc.scalar.activation(
                                ptt[:, dA:512], psc[:, dA:512], AF.Exp
                            )
                            nc.scalar.activation(
                                ptt[:, 512 + dB : 1024], psc[:, 512 + dB : 1024], AF.Exp
                            )
                        else:
                            nc.scalar.activation(
                                ptt[:, dA:1024], psc[:, dA:1024], AF.Exp
                            )
                        for c, half, d in ((cA, 0, dA), (cB, 1, dB)):
                            kc0 = c * 128
                            if kc0 + 127 > q0:
                                # only the 128 columns crossing the diagonal
                                sl = slice(half * 512 + d, half * 512 + d + 128)
                                nc.gpsimd.affine_select(
                                    out=ptt[:, sl],
                                    in_=ptt[:, sl],
                                    compare_op=ALU.is_ge,
                                    fill=0.0,
                                    base=0,
                                    channel_multiplier=-1,
                                    pattern=[[1, 128]],
                                )
                        for c, half, d in ((cA, 0, dA), (cB, 1, dB)):
                            nc.tensor.matmul(
                                pso[0:65, d:QBW],
                                lhsT=v_s[:, c, :],
                                rhs=ptt[:, half * 512 + d : half * 512 + 512],
                                start=(c == 0),
                                stop=(c == nkc - 1),
                            )
                    # evict raw attn out + sums
                    ch = h // 2
                    rr = 64 * (h % 2)
                    stg = stgp.tile([128, QBW], BF16, tag="stg")
                    nc.vector.tensor_copy(stg[0:64, :], pso[0:64, :])
                    sumr = stgp.tile([128, QBW], F32, tag="sumr")
                    nc.vector.tensor_copy(sumr[64:65, :], pso[64:65, :])
                    nc.sync.dma_start(
                        ao[rr : rr + 64, ch, qb * QBW : (qb + 1) * QBW], stg[0:64, :]
                    )
                    nc.gpsimd.dma_start(
                        sums_h[h][qb : qb + 1, :], sumr[64:65, :]
                    )

                xt_r = xt_d.rearrange("(ko p) s -> p ko s", p=128)
                # HAM warm-up: dummy matmuls keep the PE clock ramping to full
                # rate through the DMA-bound start (values unused).
                wrm = pso_p.tile([128, QBW], F32, tag="pso", name="wrm")
                for w in range(20):
                    nc.tensor.matmul(
                        wrm[:],
                        lhsT=ktdup[:, 0:128],
                        rhs=ktdup[:, 0:512],
                        start=True,
                        stop=True,
                    )
                qs_all = [qsp.tile([128, S], BF16, tag="qs", name=f"qs{i}") for i in range(HQ)]
                qs0 = qs_all[0]
                for sb in range(NSB):
                    sbc = slice(sb * SBW, (sb + 1) * SBW)
                    xblk = xp.tile([128, DCH, SBW], BF16, tag="xblk")
                    for kq in range(4):
                        if sb == 0:
                            for kc in range(4 * kq, 4 * kq + 4):
                                nc.sync.dma_start(wq_s[:, kc, :], wqkv_r[:, kc, :])
                        nc.sync.dma_start(
                            xblk[:, 4 * kq : 4 * kq + 4, :],
                            xt_r[:, 4 * kq : 4 * kq + 4, sbc],
                        )
                    if sb == 0:
                        nc.sync.dma_start(cos_s[:], cos_d[:])
                        nc.sync.dma_start(sin_s[:], sin_d[:])
                        make_identity(nc, ident[:])
                    psKV = ps1.tile([128, SBW], F32, tag="proj")
                    psA = ps1.tile([128, SBW], F32, tag="proj")
                    psB = ps1.tile([128, SBW], F32, tag="proj")
                    for ps_t, col0 in ((psKV, 256), (psA, 0), (psB, 128)):
                        for kc in range(DCH):
                            nc.tensor.matmul(
                                ps_t[:],
                                lhsT=wq_s[:, kc, col0 : col0 + 128],
                                rhs=xblk[:, kc, :],
                                start=(kc == 0),
                                stop=(kc == DCH - 1),
                            )
                    # evict k|v rows early (frees the KV slot)
                    nc.scalar.activation(kvraw[:, sbc], psKV[:], AF.Copy)

                    # RoPE on the 4 q heads (A = first-half dims, B = second)
                    tmp = tmpp.tile([128, SBW], F32, tag="tmp")
                    nc.vector.tensor_tensor(
                        outA[:, sbc], psA[:], cos_s[:, sbc], ALU.mult
                    )
                    nc.vector.tensor_tensor(tmp[:], psB[:], sin_s[:, sbc], ALU.mult)
                    nc.vector.tensor_tensor(
                        outA[:, sbc], outA[:, sbc], tmp[:], ALU.subtract
                    )
                    tmp2 = tmpp.tile([128, SBW], F32, tag="tmp")
                    nc.vector.tensor_tensor(
                        outB[:, sbc], psB[:], cos_s[:, sbc], ALU.mult
                    )
                    nc.vector.tensor_tensor(tmp2[:], psA[:], sin_s[:, sbc], ALU.mult)
                    nc.vector.tensor_tensor(
                        outB[:, sbc], outB[:, sbc], tmp2[:], ALU.add
                    )

                    # k RoPE on this s-block: kswap = [k_hi; k_lo]
                    nc.sync.dma_start(kswap[0:32, sbc], kvraw[32:64, sbc])
                    nc.sync.dma_start(kswap[32:64, sbc], kvraw[0:32, sbc])
                    nc.vector.tensor_tensor(
                        ktdup[0:64, sbc], kvraw[0:64, sbc], cos_s[0:64, sbc], ALU.mult
                    )
                    tmpk = tmpp.tile([64, SBW], F32, tag="tmpk")
                    nc.vector.tensor_tensor(
                        tmpk[:], kswap[:, sbc], sin_s[0:64, sbc], ALU.mult
                    )
                    nc.vector.tensor_tensor(
                        ktdup[0:32, sbc], ktdup[0:32, sbc], tmpk[0:32, :],
                        ALU.subtract,
                    )
                    nc.vector.tensor_tensor(
                        ktdup[32:64, sbc], ktdup[32:64, sbc], tmpk[32:64, :],
                        ALU.add,
                    )
                    nc.sync.dma_start(ktdup[64:128, sbc], ktdup[0:64, sbc])

                    # v: [64, 512] -> 4 key-chunk tiles [128, 64] via PE transpose
                    for c in range(4 * sb, 4 * sb + 4):
                        ptr = pso_p.tile([128, QBW], F32, tag="pso")
                        nc.tensor.transpose(
                            ptr[:, 0:64],
                            kvraw[64:128, c * 128 : (c + 1) * 128],
                            ident[64:128, 64:128],
                        )
                        nc.vector.tensor_copy(v_s[:, c, 0:64], ptr[:, 0:64])
                    # head-0 q stream for this s-block + interleaved unit
                    hc0 = slice(0, 32)
                    nc.sync.dma_start(qs0[0:32, sbc], outA[hc0, sbc])
                    nc.sync.dma_start(qs0[32:64, sbc], outB[hc0, sbc])
                    nc.sync.dma_start(qs0[64:96, sbc], outA[hc0, sbc])
                    nc.sync.dma_start(qs0[96:128, sbc], outB[hc0, sbc])
                    if sb >= 1:
                        emit_unit(0, sb - 1, qs0)

                # ---------------- P2: attention --------------------------------
                nc.sync.dma_start(sel_s[:], sel_d[:])
                for ch in range(2):
                    nc.sync.dma_start(wo_s[:, ch, :], wo_d[ch])

                def _defer_normalize(hh):
                    nc.vector.reciprocal_approx_accurate(
                        rcp_f32[:], sums_h[hh][:], rcp_scr[:]
                    )
                    nc.vector.tensor_copy(rcp_all[:], rcp_f32[:])
                    for qb2 in range(NQB):
                        idx2 = 2 * qb2 + (hh % 2)
                        ch2 = hh // 2
                        rr2 = 64 * (hh % 2)
                        q02 = qb2 * QBW
                        pbc = pssc.tile([128, 1024], F32, tag="psc")
                        nc.tensor.matmul(
                            pbc[:, 0:QBW],
                            lhsT=sel_s[:, idx2 * 128 : (idx2 + 1) * 128],
                            rhs=rcp_all[:],
                            start=True,
                            stop=True,
                        )
                        nc.vector.tensor_tensor(
                            ao[rr2 : rr2 + 64, ch2, q02 : q02 + QBW],
                            ao[rr2 : rr2 + 64, ch2, q02 : q02 + QBW],
                            pbc[rr2 : rr2 + 64, 0:QBW],
                            ALU.mult,
                        )

                def _normalize_one(hh, qb2):
                    nc.vector.reciprocal_approx_accurate(
                        rcp_f32[:], sums_all[:], rcp_scr[:]
                    )
                    nc.vector.tensor_copy(rcp_all[:], rcp_f32[:])
                    idx2 = 4 * hh + qb2
                    ch2 = hh // 2
                    rr2 = 64 * (hh % 2)
                    q02 = qb2 * QBW
                    pbc = pssc.tile([128, 1024], F32, tag="psc")
                    nc.tensor.matmul(
                        pbc[:, 0:QBW],
                        lhsT=sel_s[:, idx2 * 128 : (idx2 + 1) * 128],
                        rhs=rcp_all[:],
                        start=True,
                        stop=True,
                    )
                    nc.vector.tensor_tensor(
                        ao[rr2 : rr2 + 64, ch2, q02 : q02 + QBW],
                        ao[rr2 : rr2 + 64, ch2, q02 : q02 + QBW],
                        pbc[rr2 : rr2 + 64, 0:QBW],
                        ALU.mult,
                    )

                for h in range(HQ):
                    if h == 0:
                        emit_unit(0, 3, qs0)
                        continue
                    hc = slice(32 * h, 32 * h + 32)
                    qs = qs_all[h]
                    for sb in range(NSB):
                        sbc = slice(sb * SBW, (sb + 1) * SBW)
                        nc.sync.dma_start(qs[0:32, sbc], outA[hc, sbc])
                        nc.sync.dma_start(qs[32:64, sbc], outB[hc, sbc])
                        nc.sync.dma_start(qs[64:96, sbc], outA[hc, sbc])
                        nc.sync.dma_start(qs[96:128, sbc], outB[hc, sbc])
                    qbo = (3, 2, 1, 0) if h == HQ - 1 else range(NQB)
                    for qb in qbo:
                        emit_unit(h, qb, qs)
                    _defer_normalize(h - 1)
                    if h == HQ - 1:
                        _defer_normalize(h)

                # ---------------- P3: o_proj -----------------------------------
                for st in range(16):
                    for obp in range(2):
                        psy = pssc.tile([128, 1024], F32, tag="psc")
                        for oh in range(2):
                            ob = 2 * obp + oh
                            for ch in range(2):
                                nc.tensor.matmul(
                                    psy[:, oh * 512 : (oh + 1) * 512],
                                    lhsT=ao[:, ch, st * 128 : (st + 1) * 128],
                                    rhs=wo_s[:, ch, ob * 512 : (ob + 1) * 512],
                                    start=(ch == 0),
                                    stop=(ch == 1),
                                )
                        ysb = yp.tile([128, 1024], BF16, tag="ysb")
                        if obp % 2 == 0:
                            nc.scalar.activation(ysb[:], psy[:], AF.Copy)
                        else:
                            nc.vector.tensor_copy(ysb[:], psy[:])
                        eng = nc.gpsimd if st % 2 == 0 else nc.sync
                        eng.dma_start(
                            y_d[
                                st * 128 : (st + 1) * 128,
                                obp * 1024 : (obp + 1) * 1024,
                            ],
                            ysb[:],
                        )
    nc.compile()
    return nc


def _prep_inputs(x, Wq, Wk, Wv, Wo, inv_freq):
    """Host-side sharding + layout prep. Returns in_maps for the 8 cores."""
    x = np.ascontiguousarray(np.asarray(x, dtype=np.float32).reshape(S, D))
    xt = np.ascontiguousarray(x.T)  # [D, S]

    pos = np.arange(S, dtype=np.float64)
    inv = np.asarray(inv_freq, dtype=np.float64)  # [32]
    freqs = pos[None, :] * inv[:, None]  # [32, S]
    cos32 = np.cos(freqs).astype(np.float32)
    sin32 = np.sin(freqs).astype(np.float32)
    cos_tab = np.tile(cos32, (4, 1))  # [128, S]
    sin_tab = np.tile(sin32, (4, 1))
    sel = np.zeros((16, 16 * 128), dtype=np.float32)
    for qb in range(4):
        for par in range(2):
            blk = 2 * qb + par
            rr = 64 * par
            sel[qb, blk * 128 + rr : blk * 128 + rr + 64] = 1.0

    in_maps = []
    for i in range(NCORES):
        wq_l = Wq[256 * i : 256 * (i + 1)].astype(np.float32) * 0.125  # [256, D]
        wk_l = Wk[64 * i : 64 * (i + 1)].astype(np.float32)  # [64, D]
        wv_l = Wv[64 * i : 64 * (i + 1)].astype(np.float32)  # [64, D]
        # A-tile: first-half dims of the 4 heads; B-tile: second halves
        wA = np.concatenate(
            [wq_l[64 * h : 64 * h + 32] for h in range(HQ)], axis=0
        )  # [128, D]
        wB = np.concatenate(
            [wq_l[64 * h + 32 : 64 * h + 64] for h in range(HQ)], axis=0
        )
        wkv = np.concatenate([wk_l, wv_l], axis=0)  # [128, D]
        wqkv = np.ascontiguousarray(
            np.concatenate([wA, wB, wkv], axis=0).T
        )  # [D, 384]
        wo_l = Wo[:, 256 * i : 256 * (i + 1)].astype(np.float32)  # [D, 256]
        wo_t = np.ascontiguousarray(wo_l.T.reshape(2, 128, D))  # [2, 128, D]
        in_maps.append(
            {
                "xt": xt.astype(ml_dtypes.bfloat16),
                "wqkv": wqkv.astype(ml_dtypes.bfloat16),
                "wo": wo_t.astype(ml_dtypes.bfloat16),
                "cos": cos_tab.astype(ml_dtypes.bfloat16),
                "sin": sin_tab.astype(ml_dtypes.bfloat16),
                "sel": sel.astype(ml_dtypes.bfloat16),
            }
        )
    return in_maps


_NC_CACHE = None


def kernel(x, Wq, Wk, Wv, Wo, inv_freq):
    global _NC_CACHE
    if _NC_CACHE is None:
        _NC_CACHE = _build_nc()
    nc = _NC_CACHE
    in_maps = _prep_inputs(x, Wq, Wk, Wv, Wo, inv_freq)
    trace = bool(int(os.environ.get("BASS_KERNEL_TRACE", "0")))
    res = None
    last_exc = None
    for attempt in range(3):
        try:
            res = run_bass_kernel_spmd(nc, in_maps, list(range(NCORES)), trace=trace)
            break
        except Exception as e:  # transient device faults (rare) — retry
            last_exc = e
            msg = str(e)
            if "UNRECOVERABLE" in msg or "UNAVAILABLE" in msg or "Timeout" in msg:
                continue
            raise
    if res is None:
        raise last_exc
    if trace:
        kernel.last_results = res
    y = np.zeros((S, D), dtype=np.float32)
    for i in range(NCORES):
        y += res.results[i]["y"].astype(np.float32)
    return y.reshape(1, S, D)

